# revision 1
# baseline (speedup 1.0000x reference)
"""Causal MHA (B=2, S=2048, D=1024, H=16) sharded over 8 NeuronCores.

Sharding: batch x heads. Core c owns batch c//4 and heads 4*(c%4)..4*(c%4)+4
(weight rows 256*(c%4)..+256). Wq/Wk/Wv split column-parallel by head, Wo
row-parallel; the host sums the 4 partial outputs per batch.

Per-core pipeline (matmul inputs fp16, PSUM f32):
  xT [d, s] loaded straight from DRAM (host pre-transposes + pre-casts).
  Phase A: QT/KT [128, 2, S] = W-stationary matmuls over xT, RoPE fused in
    (pair-swap via P2 matmul + cos/sin DVE ops); V [s, dk] built directly
    with xT-stationary matmuls into vns[k-tile, head, 0:64] (+ones col 64
    = softmax denominator accumulator).
  Phase B per (head, 1024-wide q chunk): S^T[k,q] = K-stationary @ Q,
    P = exp(S/8) (no max subtraction; logits are O(1)), causal diag masked
    by a triangular fp16 multiply on DVE, attn^T[65, q] += V'.T @ P^T.
    Epilogue: rden = reciprocal_approx_fast(den row, via SBUF),
    broadcast across partitions through a stride-0 DRAM roundtrip,
    stackT = attn^T * rden. Oproj for each finished q chunk is interleaved
    as PE filler: out[q, o] = stackT.T @ WoT, streamed to DRAM.
"""

import sys

import numpy as np

sys.path.insert(0, "/opt/trn_rl_repo")

B, S, D, H = 2, 2048, 1024, 16
DK = D // H            # 64
NCORES = 8
CPB = 4                # cores per batch
HPC = H // CPB         # 4 heads per core
DKH = HPC * DK         # 256 local head-dim
NMT = DKH // 128       # 2 partition tiles of the local head dim
THETA = 10000.0
SCALE = 1.0 / float(np.sqrt(DK))

NT = S // 128          # 16 kr tiles
NCH = S // 1024        # 2 q chunks


def _rope_tables():
    pos = np.arange(S, dtype=np.float64)
    dim = np.arange(0, DK, 2, dtype=np.float64)
    inv_freq = 1.0 / THETA ** (dim / DK)
    angle = pos[None, :] * inv_freq[:, None]        # [DK/2, S]
    angle = np.repeat(angle, 2, axis=0)             # [DK, S] interleaved rows
    cos1, sin1 = np.cos(angle), np.sin(angle)
    cosT = np.concatenate([cos1, cos1], axis=0).astype(np.float16)  # [128, S]
    sinT = np.concatenate([sin1, sin1], axis=0).astype(np.float16)
    return cosT, sinT


def _p2t():
    # pair rotation: out[2i] = -in[2i+1]; out[2i+1] = +in[2i], per 64-row head.
    # matmul computes lhsT.T @ rhs, so pass P2^T. [128,128] covers 2 heads.
    p = np.zeros((DK, DK), dtype=np.float32)
    for i in range(DK // 2):
        p[2 * i, 2 * i + 1] = -1.0
        p[2 * i + 1, 2 * i] = 1.0
    p2 = np.zeros((128, 128), dtype=np.float32)
    p2[:DK, :DK] = p
    p2[DK:, DK:] = p
    return np.ascontiguousarray(p2.T).astype(np.float16)


def _mask_tables():
    # ident.T @ mneg accumulated into a diagonal score tile adds -240 where
    # q < k (pre-scale), so exp(s/8 - 30) == 0 for non-causal pairs
    ident = np.eye(128, dtype=np.float16)
    mneg = np.tril(np.full((128, 128), -240.0, dtype=np.float16), -1)
    return ident, mneg


def _col_pieces(qs):
    """Split [qs, 1024) into <=512-wide matmul column pieces."""
    if qs >= 512:
        return [(qs, 1024)]
    return [(qs, 512), (512, 1024)]


def _build_nc(debug=False):
    from contextlib import ExitStack

    import concourse.bass as bass
    import concourse.tile as tile
    from concourse import bacc, mybir

    fp16 = mybir.dt.float16
    bf16 = mybir.dt.bfloat16
    f32 = mybir.dt.float32
    EXP = mybir.ActivationFunctionType.Exp
    MULT = mybir.AluOpType.mult

    nc = bacc.Bacc(
        "TRN2", target_bir_lowering=False, debug=False, num_devices=NCORES
    )
    xt_d = nc.dram_tensor("xT", [D, S], fp16, kind="ExternalInput")
    wqt_d = nc.dram_tensor("wqt", [D, DKH], fp16, kind="ExternalInput")
    wkt_d = nc.dram_tensor("wkt", [D, DKH], fp16, kind="ExternalInput")
    wvt_d = nc.dram_tensor("wvt", [D, DKH], fp16, kind="ExternalInput")
    wot_d = nc.dram_tensor("wot", [DKH, D], fp16, kind="ExternalInput")
    cos_d = nc.dram_tensor("cosT", [128, S], fp16, kind="ExternalInput")
    sin_d = nc.dram_tensor("sinT", [128, S], fp16, kind="ExternalInput")
    p2t_d = nc.dram_tensor("p2t", [128, 128], fp16, kind="ExternalInput")
    ident_d = nc.dram_tensor("ident", [128, 128], fp16, kind="ExternalInput")
    mneg_d = nc.dram_tensor("mneg", [128, 128], fp16, kind="ExternalInput")
    out_d = nc.dram_tensor("out", [S, D], fp16, kind="ExternalOutput")
    if debug:
        dbg_q = nc.dram_tensor("dbg_q", [128, NMT, S], fp16, kind="ExternalOutput")
        dbg_k = nc.dram_tensor("dbg_k", [128, NMT, S], fp16, kind="ExternalOutput")
        dbg_v = nc.dram_tensor("dbg_v", [128, NT * HPC * 65], bf16, kind="ExternalOutput")
        dbg_rden = nc.dram_tensor("dbg_rden", [8, 1, 1024], f32, kind="ExternalOutput")
        dbg_rdb = nc.dram_tensor("dbg_rdb", [8, 64, 1024], f32, kind="ExternalOutput")
        dbg_st = nc.dram_tensor("dbg_st", [128, NMT, S], fp16, kind="ExternalOutput")

    with tile.TileContext(nc) as tc, ExitStack() as ctx:
        consts = ctx.enter_context(tc.tile_pool(name="consts", bufs=1))
        xtp = ctx.enter_context(tc.tile_pool(name="xt", bufs=1))
        qkp = ctx.enter_context(tc.tile_pool(name="qk", bufs=1))
        vnp = ctx.enter_context(tc.tile_pool(name="vn", bufs=1))
        stackp = ctx.enter_context(tc.tile_pool(name="stack", bufs=1))
        rawp = ctx.enter_context(tc.tile_pool(name="raw", bufs=2))
        tmpp = ctx.enter_context(tc.tile_pool(name="tmp", bufs=4))
        ptp = ctx.enter_context(tc.tile_pool(name="pt", bufs=6))
        epip = ctx.enter_context(tc.tile_pool(name="epi", bufs=2))
        dramp = ctx.enter_context(tc.tile_pool(name="dram", bufs=2, space="DRAM"))
        outp = ctx.enter_context(tc.tile_pool(name="outsb", bufs=3))

        # ---- constants (DMA order = compute priority) ----
        wq_sb = consts.tile([128, 8, DKH], fp16, tag="wq")
        wk_sb = consts.tile([128, 8, DKH], fp16, tag="wk")
        wv_sb = consts.tile([128, 8, DKH], fp16, tag="wv")
        nc.sync.dma_start(wk_sb, wkt_d[:, :].rearrange("(j p) m -> p j m", p=128))
        nc.sync.dma_start(wq_sb, wqt_d[:, :].rearrange("(j p) m -> p j m", p=128))
        nc.sync.dma_start(wv_sb, wvt_d[:, :].rearrange("(j p) m -> p j m", p=128))
        cos_sb = consts.tile([128, S], fp16, tag="cos")
        sin_sb = consts.tile([128, S], fp16, tag="sin")
        nc.sync.dma_start(cos_sb, cos_d[:, :])
        nc.sync.dma_start(sin_sb, sin_d[:, :])
        xt = xtp.tile([128, 8, S], fp16, tag="xt")
        xt_src = xt_d[:, :].rearrange("(j p) s -> p j s", p=128)
        for cch in range(4):
            sl = slice(512 * cch, 512 * (cch + 1))
            nc.sync.dma_start(xt[:, :, sl], xt_src[:, :, sl])
        p2_sb = consts.tile([128, 128], fp16, tag="p2")
        nc.sync.dma_start(p2_sb, p2t_d[:, :])
        ident_sb = consts.tile([128, 128], fp16, tag="ident")
        nc.sync.dma_start(ident_sb, ident_d[:, :])
        mneg_sb = consts.tile([128, 128], fp16, tag="mneg")
        nc.sync.dma_start(mneg_sb, mneg_d[:, :])
        wo_sb = consts.tile([128, NMT, D], fp16, tag="wo")
        nc.sync.dma_start(wo_sb, wot_d[:, :].rearrange("(k p) m -> p k m", p=128))

        qtr = qkp.tile([128, NMT, S], fp16, tag="qtr")
        ktr = qkp.tile([128, NMT, S], fp16, tag="ktr")
        vns = vnp.tile([128, NT, HPC, 65], bf16, tag="vn")
        # contiguous memset (strided single-column memset misbehaves on HW);
        # V copies then overwrite cols 0:64, leaving col 64 as the ones row
        nc.vector.memset(vns[:, :, :, :], 1.0)
        stackT = stackp.tile([128, NMT, S], fp16, tag="stack")

        # ================= phase A: projections + rope =================
        with tc.tile_pool(name="psA", bufs=3, space="PSUM") as psA:

            def emit_rot(raw, dst, mt, sl):
                # rope pair-rotation, pipelined one unit back so the PE
                # never waits on the raw copy
                rot_ps = psA.tile([128, 512], f32, tag="ps")
                nc.tensor.matmul(rot_ps, p2_sb, raw, start=True, stop=True)
                t1 = tmpp.tile([128, 512], fp16, tag="tmp")
                nc.vector.tensor_mul(t1, raw, cos_sb[:, sl])
                t2 = tmpp.tile([128, 512], fp16, tag="tmp")
                nc.vector.tensor_tensor(t2, rot_ps, sin_sb[:, sl], op=MULT)
                nc.vector.tensor_add(dst[:, mt, sl], t1, t2)

            pend_rot = None
            for cch in range(4):
                sl = slice(512 * cch, 512 * (cch + 1))
                for w_sb, dst in ((wk_sb, ktr), (wq_sb, qtr)):
                    for mt in range(NMT):
                        ps = psA.tile([128, 512], f32, tag="ps")
                        for j in range(8):
                            nc.tensor.matmul(
                                ps,
                                w_sb[:, j, 128 * mt : 128 * (mt + 1)],
                                xt[:, j, sl],
                                start=(j == 0),
                                stop=(j == 7),
                            )
                        raw = rawp.tile([128, 512], fp16, tag="raw")
                        nc.scalar.copy(raw, ps)
                        if pend_rot is not None:
                            emit_rot(*pend_rot)
                        pend_rot = (raw, dst, mt, sl)
                for st in range(4):  # V for the chunk's 4 seq tiles
                    t_g = 4 * cch + st
                    ssl = slice(512 * cch + 128 * st, 512 * cch + 128 * (st + 1))
                    vps = psA.tile([128, DKH], f32, tag="ps")
                    for j in range(8):
                        nc.tensor.matmul(
                            vps,
                            xt[:, j, ssl],
                            wv_sb[:, j, :],
                            start=(j == 0),
                            stop=(j == 7),
                        )
                    nc.vector.tensor_copy(
                        vns[:, t_g, :, 0:64],
                        vps[:, :].rearrange("p (h d) -> p h d", h=HPC),
                    )
                    if pend_rot is not None:
                        emit_rot(*pend_rot)
                        pend_rot = None
            if pend_rot is not None:
                emit_rot(*pend_rot)

        if debug:
            nc.sync.dma_start(dbg_q[:, :, :], qtr[:, :, :])
            nc.sync.dma_start(dbg_k[:, :, :], ktr[:, :, :])
            nc.sync.dma_start(dbg_v[:, :], vns[:, :, :, :].rearrange("p a b c -> p (a b c)"))

        # ================= phase B: attention =================
        with (
            tc.tile_pool(name="ps_sc", bufs=2, space="PSUM") as ps_sc,
            tc.tile_pool(name="ps_at", bufs=1, space="PSUM") as ps_at,
            tc.tile_pool(name="ps_po", bufs=2, space="PSUM") as ps_po,
        ):
            def oproj(qt_i):
                osb = outp.tile([128, D], fp16, tag="osb")
                for oc in range(2):
                    po = ps_po.tile([128, 512], f32, tag="po")
                    for mt2 in range(NMT):
                        nc.tensor.matmul(
                            po,
                            stackT[:, mt2, 128 * qt_i : 128 * (qt_i + 1)],
                            wo_sb[:, mt2, 512 * oc : 512 * (oc + 1)],
                            start=(mt2 == 0),
                            stop=(mt2 == NMT - 1),
                        )
                    nc.vector.tensor_copy(osb[:, 512 * oc : 512 * (oc + 1)], po)
                nc.sync.dma_start(out_d[128 * qt_i : 128 * (qt_i + 1), :], osb)
            for cch in range(NCH):  # q chunks of 1024
                qbase = 1024 * cch
                n_kt = min(NT, 8 * (cch + 1))
                for h in range(HPC):
                    mt = h // 2
                    hsl = slice(64 * (h % 2), 64 * (h % 2) + 64)
                    at_ps = ps_at.tile([65, 1024], f32, tag="at")

                    def emit_pv(t, pt, qs):
                        # PSUM accumulation-group granularity is the 2KB bank
                        # (512 f32 cols). stop=True must land on each bank's
                        # final writer (the tile whose diagonal block ends it).
                        for lo, hi in _col_pieces(qs):
                            nc.tensor.matmul(
                                at_ps[:, lo:hi],
                                vns[:, t, h, 0:65],
                                pt[:, lo:hi],
                                start=(t == 0),
                                stop=(qs == hi - 128),
                            )

                    from collections import deque

                    pend_q = deque()  # software-pipeline PV two kr-tiles back
                    for t in range(n_kt):
                        qs = max(128 * t - qbase, 0)
                        diag = 128 * t >= qbase
                        sc_ps = ps_sc.tile([128, 1024], f32, tag="sc")
                        for lo, hi in _col_pieces(qs):
                            nc.tensor.matmul(
                                sc_ps[:, lo:hi],
                                ktr[hsl, mt, 128 * t : 128 * (t + 1)],
                                qtr[hsl, mt, qbase + lo : qbase + hi],
                                start=True,
                                stop=not (diag and lo == qs),
                            )
                        if diag:
                            # causal mask: add -240 (pre-scale) where q < k
                            nc.tensor.matmul(
                                sc_ps[:, qs : qs + 128],
                                ident_sb,
                                mneg_sb,
                                start=False,
                                stop=True,
                            )
                        pt = ptp.tile([128, 1024], bf16, tag="pt")
                        # bf16 P tiles: fp32-range exponent, exp cannot
                        # overflow; bf16 matmul runs at fp16 speed
                        nc.scalar.activation(
                            pt[:, qs:1024], sc_ps[:, qs:1024], EXP, scale=SCALE
                        )
                        pend_q.append((t, pt, qs))
                        if len(pend_q) > 2:
                            emit_pv(*pend_q.popleft())
                    while pend_q:
                        emit_pv(*pend_q.popleft())
                    # epilogue: normalize attn^T by the ones-row denom.
                    # 1-partition den copy + direct PSUM reads (HW-proven ops)
                    den_sb = epip.tile([1, 1024], f32, tag="den")
                    nc.vector.tensor_copy(den_sb, at_ps[64:65, :])
                    at64 = epip.tile([64, 1024], f32, tag="at64")
                    nc.vector.tensor_copy(at64, at_ps[0:64, :])
                    rden = epip.tile([1, 1024], f32, tag="rden")
                    nc.vector.reciprocal_approx_fast(out=rden, in_=den_sb)
                    den_dr = dramp.tile([1, 1024], f32, tag="dendr")
                    nc.sync.dma_start(den_dr, rden)
                    rdb = epip.tile([64, 1024], f32, tag="rdb")
                    dr_ap = den_dr[:, :]
                    den_src = bass.AP(
                        tensor=dr_ap.tensor,
                        offset=dr_ap.offset,
                        ap=[[0, 64]] + dr_ap.ap[1:],
                    )
                    nc.sync.dma_start(rdb, den_src)
                    nc.vector.tensor_tensor(
                        stackT[hsl, mt, qbase : qbase + 1024],
                        at64,
                        rdb,
                        op=MULT,
                    )
                    if debug:
                        di = h * NCH + cch
                        nc.sync.dma_start(dbg_rden[di, :, :], rden)
                        nc.sync.dma_start(dbg_rdb[di, :, :], rdb)
                # output projection for the finished q chunk (PE filler)
                for qt_i in range(8 * cch, 8 * (cch + 1)):
                    oproj(qt_i)
        if debug:
            nc.sync.dma_start(dbg_st[:, :, :], stackT[:, :, :])

    nc.compile()
    return nc


_NC_CACHE = None


def _in_maps(x, Wq, Wk, Wv, Wo):
    cosT, sinT = _rope_tables()
    p2t = _p2t()
    ident, mneg = _mask_tables()
    Wq, Wk, Wv, Wo = (np.asarray(w, dtype=np.float32) for w in (Wq, Wk, Wv, Wo))
    x = np.asarray(x, dtype=np.float32)
    xts = [
        np.ascontiguousarray(x[b].T.astype(np.float16)) for b in range(B)
    ]
    in_maps = []
    for c in range(NCORES):
        b = c // CPB
        rows = slice(DKH * (c % CPB), DKH * (c % CPB + 1))
        in_maps.append(
            {
                "xT": xts[b],
                "wqt": np.ascontiguousarray(Wq[rows, :].T.astype(np.float16)),
                "wkt": np.ascontiguousarray(Wk[rows, :].T.astype(np.float16)),
                "wvt": np.ascontiguousarray(Wv[rows, :].T.astype(np.float16)),
                "wot": np.ascontiguousarray(Wo[:, rows].T.astype(np.float16)),
                "cosT": cosT,
                "sinT": sinT,
                "p2t": p2t,
                "ident": ident,
                "mneg": mneg,
            }
        )
    return in_maps


def kernel(x, Wq, Wk, Wv, Wo):
    global _NC_CACHE
    from concourse.bass_utils import run_bass_kernel_spmd

    if _NC_CACHE is None:
        _NC_CACHE = _build_nc()
    nc = _NC_CACHE

    in_maps = _in_maps(x, Wq, Wk, Wv, Wo)
    res = run_bass_kernel_spmd(nc, in_maps, core_ids=list(range(NCORES)))
    out = np.zeros((B, S, D), dtype=np.float32)
    for c, r in enumerate(res.results):
        out[c // CPB] += r["out"].astype(np.float32)
    return out



# revision 7
# speedup vs baseline: 1.0296x; 1.0296x over previous
"""Causal MHA (B=2, S=2048, D=1024, H=16) sharded over 8 NeuronCores.

Sharding: batch x heads. Core c owns batch c//4 and heads 4*(c%4)..4*(c%4)+4
(weight rows 256*(c%4)..+256). Wq/Wk/Wv split column-parallel by head, Wo
row-parallel; the host sums the 4 partial outputs per batch.

Per-core pipeline (matmul inputs fp16, PSUM f32):
  Host pre-lays every tensor out in its exact SBUF layout so all DMAs are
  contiguous; small constants are queued first so the PE starts at ~7us.
  Projections run K(all chunks) -> V(all) -> Q(chunk 0,1); attention for
  q-chunk 0 is then interleaved with Q(chunk 2,3) so ACT's exp stream (the
  secondary bottleneck) starts while the PE is still doing projections.
  QT/KT [128, 2, S] = W-stationary matmuls over xT with RoPE fused
  (pair-swap via P2 matmul + cos/sin DVE ops). V rows are built directly
  in vns[k-tile, head, 0:64]; columns 64:128 are left at the memset value
  1.0, so the PV matmul (full 128-wide stationary) replicates the softmax
  denominator into PSUM partitions 64:128 for free - the epilogue is just
  reciprocal_approx_fast on [64,1024] + one tensor_tensor multiply, with
  no cross-partition broadcast needed.
  Per (head, 1024-wide q chunk): S^T[k,q] = K-stationary @ Q, P = exp(S/8)
  (no max subtraction; logits are O(1)), causal diag masked by an
  ident.T@mneg accumulation (-240 pre-scale). attn^T[128, q] += V'.T @ P^T.
  Oproj per finished q chunk is interleaved as PE filler, streamed to DRAM.
"""

import sys
from collections import deque

import numpy as np

sys.path.insert(0, "/opt/trn_rl_repo")

B, S, D, H = 2, 2048, 1024, 16
DK = D // H            # 64
NCORES = 8
CPB = 4                # cores per batch
HPC = H // CPB         # 4 heads per core
DKH = HPC * DK         # 256 local head-dim
NMT = DKH // 128       # 2 partition tiles of the local head dim
THETA = 10000.0
SCALE = 1.0 / float(np.sqrt(DK))

NT = S // 128          # 16 kr tiles
NCH = S // 1024        # 2 q chunks
NXC = 4                # x chunks of 512
XCW = S // NXC         # 512


def _rope_tables():
    pos = np.arange(S, dtype=np.float64)
    dim = np.arange(0, DK, 2, dtype=np.float64)
    inv_freq = 1.0 / THETA ** (dim / DK)
    angle = pos[None, :] * inv_freq[:, None]        # [DK/2, S]
    angle = np.repeat(angle, 2, axis=0)             # [DK, S] interleaved rows
    cos1, sin1 = np.cos(angle), np.sin(angle)
    cosT = np.concatenate([cos1, cos1], axis=0).astype(np.float16)  # [128, S]
    sinT = np.concatenate([sin1, sin1], axis=0).astype(np.float16)
    return cosT, sinT


def _p2t():
    # pair rotation: out[2i] = -in[2i+1]; out[2i+1] = +in[2i], per 64-row head.
    # matmul computes lhsT.T @ rhs, so pass P2^T. [128,128] covers 2 heads.
    p = np.zeros((DK, DK), dtype=np.float32)
    for i in range(DK // 2):
        p[2 * i, 2 * i + 1] = -1.0
        p[2 * i + 1, 2 * i] = 1.0
    p2 = np.zeros((128, 128), dtype=np.float32)
    p2[:DK, :DK] = p
    p2[DK:, DK:] = p
    return np.ascontiguousarray(p2.T).astype(np.float16)


def _mask_tables():
    # ident.T @ mneg accumulated into a diagonal score tile adds -240 where
    # q < k (pre-scale), so exp(s/8 - 30) == 0 for non-causal pairs
    ident = np.eye(128, dtype=np.float16)
    mneg = np.tril(np.full((128, 128), -240.0, dtype=np.float16), -1)
    return ident, mneg


def _col_pieces(qs):
    """Split [qs, 1024) into <=512-wide matmul column pieces."""
    if qs >= 512:
        return [(qs, 1024)]
    return [(qs, 512), (512, 1024)]


def _build_nc(debug=False):
    from contextlib import ExitStack

    import concourse.bass as bass  # noqa: F401
    import concourse.tile as tile
    from concourse import bacc, mybir

    NDBG = NCH * HPC

    fp16 = mybir.dt.float16
    bf16 = mybir.dt.bfloat16
    f32 = mybir.dt.float32
    EXP = mybir.ActivationFunctionType.Exp
    MULT = mybir.AluOpType.mult

    nc = bacc.Bacc(
        "TRN2", target_bir_lowering=False, debug=False, num_devices=NCORES
    )
    # all inputs pre-laid-out by the host in exact SBUF order -> contiguous DMA
    xt_d = nc.dram_tensor("xT", [NXC, 128, 8 * XCW], fp16, kind="ExternalInput")
    wqt_d = nc.dram_tensor("wqt", [128, 8 * DKH], fp16, kind="ExternalInput")
    wkt_d = nc.dram_tensor("wkt", [128, 8 * DKH], fp16, kind="ExternalInput")
    wvt_d = nc.dram_tensor("wvt", [128, 8 * DKH], fp16, kind="ExternalInput")
    wot_d = nc.dram_tensor("wot", [128, NMT * D], fp16, kind="ExternalInput")
    cos_d = nc.dram_tensor("cosT", [128, S], fp16, kind="ExternalInput")
    sin_d = nc.dram_tensor("sinT", [128, S], fp16, kind="ExternalInput")
    p2t_d = nc.dram_tensor("p2t", [128, 128], fp16, kind="ExternalInput")
    ident_d = nc.dram_tensor("ident", [128, 128], fp16, kind="ExternalInput")
    mneg_d = nc.dram_tensor("mneg", [128, 128], fp16, kind="ExternalInput")
    out_d = nc.dram_tensor("out", [S, D], fp16, kind="ExternalOutput")
    if debug:
        dbg_q = nc.dram_tensor("dbg_q", [128, NMT, S], fp16, kind="ExternalOutput")
        dbg_k = nc.dram_tensor("dbg_k", [128, NMT, S], fp16, kind="ExternalOutput")
        dbg_v = nc.dram_tensor(
            "dbg_v", [128, NT * HPC * 128], bf16, kind="ExternalOutput"
        )
        dbg_den = nc.dram_tensor(
            "dbg_den", [NDBG, 64, 1024], f32, kind="ExternalOutput"
        )
        dbg_rdb = nc.dram_tensor(
            "dbg_rdb", [NDBG, 64, 1024], f32, kind="ExternalOutput"
        )
        dbg_st = nc.dram_tensor("dbg_st", [128, NMT, S], fp16, kind="ExternalOutput")

    with tile.TileContext(nc) as tc, ExitStack() as ctx:
        consts = ctx.enter_context(tc.tile_pool(name="consts", bufs=1))
        xtp = ctx.enter_context(tc.tile_pool(name="xt", bufs=1))
        qkp = ctx.enter_context(tc.tile_pool(name="qk", bufs=1))
        vnp = ctx.enter_context(tc.tile_pool(name="vn", bufs=1))
        stackp = ctx.enter_context(tc.tile_pool(name="stack", bufs=1))
        rawp = ctx.enter_context(tc.tile_pool(name="raw", bufs=2))
        tmpp = ctx.enter_context(tc.tile_pool(name="tmp", bufs=4))
        ptp = ctx.enter_context(tc.tile_pool(name="pt", bufs=6))
        epip = ctx.enter_context(tc.tile_pool(name="epi", bufs=2))
        outp = ctx.enter_context(tc.tile_pool(name="outsb", bufs=3))

        # ---- constants + x (DMA order = need order; tiny consts first) ----
        p2_sb = consts.tile([128, 128], fp16, tag="p2")
        nc.sync.dma_start(p2_sb, p2t_d[:, :])
        wk_sb = consts.tile([128, 8, DKH], fp16, tag="wk")
        wq_sb = consts.tile([128, 8, DKH], fp16, tag="wq")
        wv_sb = consts.tile([128, 8, DKH], fp16, tag="wv")
        nc.sync.dma_start(wk_sb, wkt_d[:, :].rearrange("p (j m) -> p j m", j=8))
        cos_sb = consts.tile([128, S], fp16, tag="cos")
        sin_sb = consts.tile([128, S], fp16, tag="sin")
        nc.sync.dma_start(cos_sb, cos_d[:, :])
        nc.sync.dma_start(sin_sb, sin_d[:, :])
        xt = xtp.tile([128, NXC, 8, XCW], fp16, tag="xt")
        nc.sync.dma_start(
            xt[:, 0], xt_d[0, :, :].rearrange("p (j s) -> p j s", j=8)
        )
        nc.sync.dma_start(
            xt[:, 1], xt_d[1, :, :].rearrange("p (j s) -> p j s", j=8)
        )
        nc.sync.dma_start(wv_sb, wvt_d[:, :].rearrange("p (j m) -> p j m", j=8))
        nc.sync.dma_start(
            xt[:, 2], xt_d[2, :, :].rearrange("p (j s) -> p j s", j=8)
        )
        nc.sync.dma_start(
            xt[:, 3], xt_d[3, :, :].rearrange("p (j s) -> p j s", j=8)
        )
        nc.sync.dma_start(wq_sb, wqt_d[:, :].rearrange("p (j m) -> p j m", j=8))
        ident_sb = consts.tile([128, 128], fp16, tag="ident")
        nc.sync.dma_start(ident_sb, ident_d[:, :])
        mneg_sb = consts.tile([128, 128], fp16, tag="mneg")
        nc.sync.dma_start(mneg_sb, mneg_d[:, :])
        wo_sb = consts.tile([128, NMT, D], fp16, tag="wo")
        nc.sync.dma_start(wo_sb, wot_d[:, :].rearrange("p (k m) -> p k m", k=NMT))

        qtr = qkp.tile([128, NMT, S], fp16, tag="qtr")
        ktr = qkp.tile([128, NMT, S], fp16, tag="ktr")
        # V rows in cols 0:64; cols 64:128 stay 1.0 so the PV matmul writes
        # the softmax denominator, already broadcast, into PSUM rows 64:128
        vns = vnp.tile([128, NT, HPC, 128], bf16, tag="vn")
        nc.vector.memset(vns[:, :, :, :], 1.0)
        stackT = stackp.tile([128, NMT, S], fp16, tag="stack")

        # PSUM pools for the whole kernel: 2 + 4 + 2 = 8 banks
        psA = ctx.enter_context(tc.tile_pool(name="psA", bufs=2, space="PSUM"))
        ps_sc = ctx.enter_context(tc.tile_pool(name="ps_sc", bufs=2, space="PSUM"))
        ps_at = ctx.enter_context(tc.tile_pool(name="ps_at", bufs=1, space="PSUM"))

        # ================= phase A helpers =================
        def emit_rot(raw, dst, mt, sl):
            # rope pair-rotation for one [128, 512] projection group
            rot_ps = psA.tile([128, XCW], f32, tag="ps")
            nc.tensor.matmul(rot_ps, p2_sb, raw, start=True, stop=True)
            t1 = tmpp.tile([128, XCW], fp16, tag="tmp")
            nc.vector.tensor_mul(t1, raw, cos_sb[:, sl])
            t2 = tmpp.tile([128, XCW], fp16, tag="tmp")
            nc.vector.tensor_tensor(t2, rot_ps, sin_sb[:, sl], op=MULT)
            nc.vector.tensor_add(dst[:, mt, sl], t1, t2)

        def proj_chunk(w_sb, dst, cch):
            # both mt groups first, rope rotations after (raw copies overlap)
            sl = slice(XCW * cch, XCW * (cch + 1))
            raws = []
            for mt in range(NMT):
                ps = psA.tile([128, XCW], f32, tag="ps")
                for j in range(8):
                    nc.tensor.matmul(
                        ps,
                        w_sb[:, j, 128 * mt : 128 * (mt + 1)],
                        xt[:, cch, j, :],
                        start=(j == 0),
                        stop=(j == 7),
                    )
                raw = rawp.tile([128, XCW], fp16, tag="raw")
                nc.scalar.copy(raw, ps)
                raws.append((raw, mt))
            for raw, mt in raws:
                emit_rot(raw, dst, mt, sl)

        def v_chunk(cch):
            for st in range(4):  # the chunk's 4 seq tiles
                t_g = 4 * cch + st
                vps = psA.tile([128, DKH], f32, tag="ps")
                for j in range(8):
                    nc.tensor.matmul(
                        vps,
                        xt[:, cch, j, 128 * st : 128 * (st + 1)],
                        wv_sb[:, j, :],
                        start=(j == 0),
                        stop=(j == 7),
                    )
                nc.vector.tensor_copy(
                    vns[:, t_g, :, 0:64],
                    vps[:, :].rearrange("p (h d) -> p h d", h=HPC),
                )

        # ================= phase B helpers =================
        def bhead(cch, h):
            qbase = 1024 * cch
            n_kt = min(NT, 8 * (cch + 1))
            mt = h // 2
            hsl = slice(64 * (h % 2), 64 * (h % 2) + 64)
            at_ps = ps_at.tile([128, 1024], f32, tag="at")

            def emit_pv(t, pt, qs):
                # PSUM accumulation-group granularity is the 2KB bank
                # (512 f32 cols). stop=True must land on each bank's
                # final writer (the tile whose diagonal block ends it).
                for lo, hi in _col_pieces(qs):
                    nc.tensor.matmul(
                        at_ps[:, lo:hi],
                        vns[:, t, h, :],
                        pt[:, lo:hi],
                        start=(t == 0),
                        stop=(qs == hi - 128),
                    )

            pend_q = deque()  # software-pipeline PV two kr-tiles back
            for t in range(n_kt):
                qs = max(128 * t - qbase, 0)
                diag = 128 * t >= qbase
                sc_ps = ps_sc.tile([128, 1024], f32, tag="sc")
                for lo, hi in _col_pieces(qs):
                    nc.tensor.matmul(
                        sc_ps[:, lo:hi],
                        ktr[hsl, mt, 128 * t : 128 * (t + 1)],
                        qtr[hsl, mt, qbase + lo : qbase + hi],
                        start=True,
                        stop=not (diag and lo == qs),
                    )
                if diag:
                    # causal mask: add -240 (pre-scale) where q < k
                    nc.tensor.matmul(
                        sc_ps[:, qs : qs + 128],
                        ident_sb,
                        mneg_sb,
                        start=False,
                        stop=True,
                    )
                pt = ptp.tile([128, 1024], bf16, tag="pt")
                # bf16 P tiles: fp32-range exponent, exp cannot
                # overflow; bf16 matmul runs at fp16 speed
                nc.scalar.activation(
                    pt[:, qs:1024], sc_ps[:, qs:1024], EXP, scale=SCALE
                )
                pend_q.append((t, pt, qs))
                if len(pend_q) > 2:
                    emit_pv(*pend_q.popleft())
            while pend_q:
                emit_pv(*pend_q.popleft())
            # epilogue: PSUM rows 64:128 already hold the denominator
            # broadcast across partitions (ones-columns of vns); recip reads
            # via SBUF (HW-proven), not straight from PSUM
            den_sb = epip.tile([64, 1024], f32, tag="den")
            nc.vector.tensor_copy(den_sb, at_ps[64:128, :])
            rdb = epip.tile([64, 1024], f32, tag="rdb")
            nc.vector.reciprocal_approx_fast(out=rdb, in_=den_sb)
            nc.vector.tensor_tensor(
                stackT[hsl, mt, qbase : qbase + 1024],
                at_ps[0:64, :],
                rdb,
                op=MULT,
            )
            if debug:
                di = h * NCH + cch
                nc.sync.dma_start(dbg_den[di, :, :], den_sb)
                nc.sync.dma_start(dbg_rdb[di, :, :], rdb)

        def oproj(qt_i):
            osb = outp.tile([128, D], fp16, tag="osb")
            for oc in range(2):
                po = psA.tile([128, 512], f32, tag="ps")
                for mt2 in range(NMT):
                    nc.tensor.matmul(
                        po,
                        stackT[:, mt2, 128 * qt_i : 128 * (qt_i + 1)],
                        wo_sb[:, mt2, 512 * oc : 512 * (oc + 1)],
                        start=(mt2 == 0),
                        stop=(mt2 == NMT - 1),
                    )
                nc.vector.tensor_copy(osb[:, 512 * oc : 512 * (oc + 1)], po)
            nc.sync.dma_start(out_d[128 * qt_i : 128 * (qt_i + 1), :], osb)

        # ================= emission schedule =================
        for cch in range(NXC):
            proj_chunk(wk_sb, ktr, cch)
        for cch in range(NXC):
            v_chunk(cch)
        proj_chunk(wq_sb, qtr, 0)
        proj_chunk(wq_sb, qtr, 1)
        # attention q-chunk 0 interleaved with the remaining Q projections
        bhead(0, 0)
        proj_chunk(wq_sb, qtr, 2)
        bhead(0, 1)
        proj_chunk(wq_sb, qtr, 3)
        bhead(0, 2)
        bhead(0, 3)
        for qt_i in range(8):
            oproj(qt_i)
        for h in range(HPC):
            bhead(1, h)
        for qt_i in range(8, 16):
            oproj(qt_i)
        if debug:
            nc.sync.dma_start(dbg_q[:, :, :], qtr[:, :, :])
            nc.sync.dma_start(dbg_k[:, :, :], ktr[:, :, :])
            nc.sync.dma_start(
                dbg_v[:, :], vns[:, :, :, :].rearrange("p a b c -> p (a b c)")
            )
            nc.sync.dma_start(dbg_st[:, :, :], stackT[:, :, :])

    nc.compile()
    return nc


_NC_CACHE = None


def _in_maps(x, Wq, Wk, Wv, Wo):
    cosT, sinT = _rope_tables()
    p2t = _p2t()
    ident, mneg = _mask_tables()
    Wq, Wk, Wv, Wo = (np.asarray(w, dtype=np.float32) for w in (Wq, Wk, Wv, Wo))
    x = np.asarray(x, dtype=np.float32)
    # x^T chunk-major: [cch, p, j, s] so every DMA is fully contiguous
    xts = []
    for b in range(B):
        xt = x[b].T.astype(np.float16)                       # [D, S]
        xt = xt.reshape(8, 128, NXC, XCW).transpose(2, 1, 0, 3)
        xts.append(np.ascontiguousarray(xt.reshape(NXC, 128, 8 * XCW)))

    def wlay(w):  # [D, DKH] -> [128, 8*DKH] in (p, j, m) order
        w = w.reshape(8, 128, DKH).transpose(1, 0, 2)
        return np.ascontiguousarray(w.reshape(128, 8 * DKH).astype(np.float16))

    in_maps = []
    for c in range(NCORES):
        b = c // CPB
        rows = slice(DKH * (c % CPB), DKH * (c % CPB + 1))
        wol = Wo[:, rows].T.reshape(NMT, 128, D).transpose(1, 0, 2)
        in_maps.append(
            {
                "xT": xts[b],
                "wqt": wlay(Wq[rows, :].T),
                "wkt": wlay(Wk[rows, :].T),
                "wvt": wlay(Wv[rows, :].T),
                "wot": np.ascontiguousarray(
                    wol.reshape(128, NMT * D).astype(np.float16)
                ),
                "cosT": cosT,
                "sinT": sinT,
                "p2t": p2t,
                "ident": ident,
                "mneg": mneg,
            }
        )
    return in_maps


def kernel(x, Wq, Wk, Wv, Wo):
    global _NC_CACHE
    from concourse.bass_utils import run_bass_kernel_spmd

    if _NC_CACHE is None:
        _NC_CACHE = _build_nc()
    nc = _NC_CACHE

    in_maps = _in_maps(x, Wq, Wk, Wv, Wo)
    res = run_bass_kernel_spmd(nc, in_maps, core_ids=list(range(NCORES)))
    out = np.zeros((B, S, D), dtype=np.float32)
    for c, r in enumerate(res.results):
        out[c // CPB] += r["out"].astype(np.float32)
    return out
